# revision 45
# baseline (speedup 1.0000x reference)
"""Trainium2 Bass kernel for nn_AdaptiveTransformerModel (gated multi-head
self-attention with per-head scalar normalization), distributed over 8
NeuronCores via head parallelism + AllToAll.

v2: host pre-transposes X (plain strided DMA loads instead of xbar
DMA-transposes), projection chunks are interleaved into the attention
stream one batch ahead so the QKV matmuls hide under the exp (ACT)
bottleneck, PSUM is re-budgeted to 8 banks (S ring 4 / O pair 2 / shared
work ring 2), per-head stats ride a single deferred norms pass (one sqrt
table switch), and the tail overlaps stats + AllGather with the second
AllToAll piece before the final projection.

Per-core computation (2 heads, all batches), bf16 matmul path / fp32 stats:
  Q^T/K^T/V^T = (X @ W{q,k,v} + b).T [128=(2 heads x 64hd), B*T] from
  host-transposed X^T tiles. Per (batch, q-chunk): S^T = K @ Q^T as a
  row-tiled pair into one 2-bank PSUM tile, E = exp(S^T/8) in one ACT pass,
  O^T[65] accumulated as [V | ones].T @ E (row 64 = softmax denominators);
  the k-loop is software-pipelined (S(t+1) before O(t)). Per-chunk sumsq
  rows via a block-diag ones matmul; sums/sumsq bounce through DRAM for
  the batched reciprocal / deferred norms. s_h = 1/max(mean norm, 1e-5) is
  AllGathered and folded into the received A2A activations; final matmul
  P^T = Wo_all^T @ (s*G) + bo (gate/H folded into Wo on the host).
"""
import os
import sys

import numpy as np

for _p in ("/root/.axon_site", "/root/.axon_site/_ro/trn_rl_repo", "/opt/trn_rl_repo"):
    if os.path.isdir(_p) and _p not in sys.path:
        sys.path.append(_p)

import ml_dtypes
import concourse.bass as bass
import concourse.bacc as bacc
import concourse.mybir as mybir
import concourse.tile as tile
from concourse import bass_utils
from concourse.bass import ts
from concourse.masks import make_identity

f32 = mybir.dt.float32
f32r = mybir.dt.float32r
bf16 = mybir.dt.bfloat16
AF = mybir.ActivationFunctionType
ALU = mybir.AluOpType
BF16NP = ml_dtypes.bfloat16

# problem shapes (hardcoded per harness contract)
B, T, D, H = 4, 2048, 1024, 16
HD = 64
NCORES = 8


class Cfg:
    def __init__(self, B=B, T=T, D=D, ncores=NCORES):
        self.B, self.T, self.D, self.ncores = B, T, D, ncores
        self.RT = B * T                  # flattened rows
        self.RSLC = self.RT // ncores    # output row slice per core
        self.DCH = D // 128              # contraction chunks for D
        self.TQ = min(512, self.RSLC // 2, T)  # q-chunk width
        self.NQC = T // self.TQ          # q-chunks per batch
        self.NKT = T // 128              # k-tiles per batch
        self.NCH = self.B * self.NQC     # total q-chunks
        self.NCP = self.NCH // 2         # chunks per parity
        assert T % self.TQ == 0 and D % 128 == 0 and self.RT % ncores == 0
        assert self.RSLC // self.TQ == 2 and self.TQ % 128 == 0


def build_body(ctx, tc, cfg, xt_d, wq, wk, wv, bq, bk, bv, wo, bo, out):
    nc = tc.nc
    DCH, TQ, NKT, NQC, RT = cfg.DCH, cfg.TQ, cfg.NKT, cfg.NQC, cfg.RT
    NCP, NHALF = cfg.NCP, cfg.RSLC // 2
    HCH = (128 * cfg.ncores) // 128

    constp = ctx.enter_context(tc.tile_pool(name="const", bufs=1))
    ident = constp.tile([128, 128], f32)
    make_identity(nc, ident[:])
    ident_bf = constp.tile([128, 128], bf16)
    nc.vector.tensor_copy(ident_bf[:], ident[:])
    ones_f32 = constp.tile([128, 1], f32)
    nc.vector.memset(ones_f32[:], 1.0)
    # block-diag ones: col0 = rows 0-63, col1 = rows 64-127 (per-head sums)
    ones2 = constp.tile([128, 2], bf16)
    nc.vector.memset(ones2[:], 0.0)
    nc.vector.memset(ones2[0:64, 0:1], 1.0)
    nc.vector.memset(ones2[64:128, 1:2], 1.0)
    ones33 = constp.tile([33, 2], bf16)
    nc.vector.memset(ones33[:], 0.0)
    nc.vector.memset(ones33[0:1, 0:1], 1.0)
    nc.vector.memset(ones33[32:33, 1:2], 1.0)
    # per-head sumsq weights landing rows 0/32 of a 33-row tile
    ones2_33 = constp.tile([128, 33], bf16)
    nc.vector.memset(ones2_33[:], 0.0)
    nc.vector.memset(ones2_33[0:64, 0:1], 1.0)
    nc.vector.memset(ones2_33[64:128, 32:33], 1.0)
    zero_row = constp.tile([1, 512], f32)
    nc.vector.memset(zero_row[:], 0.0)
    bq_sb = constp.tile([128, 1], f32)
    bk_sb = constp.tile([128, 1], f32)
    bv_sb = constp.tile([128, 1], f32)
    nc.sync.dma_start(bq_sb[:], bq[:, None])
    nc.sync.dma_start(bk_sb[:], bk[:, None])
    nc.sync.dma_start(bv_sb[:], bv[:, None])
    bo_sb = constp.tile([128, DCH], f32)
    nc.sync.dma_start(bo_sb[:], bo.rearrange("(c p) -> p c", p=128))
    scr = constp.tile([128, 1], f32)
    # dummy exp to pull the ACT exp table load into the prologue
    nc.scalar.activation(scr[:], ones_f32[:], AF.Exp)

    # persistent SBUF
    o_all, free_oa = tc.tile([128, RT], bf16, name="o_all")
    qt_all, free_qt = tc.tile([128, RT], bf16, name="qt_all")
    kt_all, free_kt = tc.tile([128, RT], bf16, name="kt_all")
    vaug, free_va = tc.tile([128, cfg.B, NKT, 2, 65], bf16, name="vaug")
    nc.vector.tensor_copy(
        vaug[:, :, :, :, 64:65],
        ones_f32[:].to_broadcast((128, cfg.B, NKT, 2, 1)))

    dramp = ctx.enter_context(tc.tile_pool(name="dram", bufs=1, space="DRAM"))
    sum_dram = dramp.tile([2, 2, NCP, TQ], f32)   # [hl, par, ci, q] softmax sums
    nrm_dram = dramp.tile([2, 2, NCP, TQ], f32)   # sumsq rows
    rec_dram = dramp.tile([2, 2, NCP, TQ], f32)   # reciprocals of sums
    ag_in = dramp.tile([8], f32)
    ag_out = dramp.tile([8 * cfg.ncores], f32, addr_space="Shared")
    rec15_dram = dramp.tile([2, TQ], f32)   # final chunk's reciprocals
    # slot payload = [o columns | 8 extra cols]; piece 1's extra cols carry
    # this core's two head scales (rows 0/1) so no separate AllGather is
    # needed -- every core receives all 16 scales with the data
    NHP = NHALF + 8
    a2a_in = [dramp.tile([cfg.ncores, 128, NHP], bf16, name=f"a2a_in{h}")
              for h in range(2)]
    a2a_out = [dramp.tile([cfg.ncores, 128, NHP], bf16, name=f"a2a_out{h}")
               for h in range(2)]
    # pre-zero the final chunk's norm slots: its contribution is added
    # on-chip, so the deferred-norms loads never wait on chunk NCH-1
    for hl in range(2):
        nc.sync.dma_start(nrm_dram[hl, 1, NCP - 1, :][None, :],
                          zero_row[0:1, 0:TQ])
        nc.sync.dma_start(rec_dram[hl, 1, NCP - 1, :][None, :],
                          zero_row[0:1, 0:TQ])
    # zero-fill the extra payload columns (only piece 1 rows 0/1 get real
    # data later; the rest must be finite for the transfer)
    zcols = constp.tile([128, 8], bf16)
    nc.vector.memset(zcols[:], 0.0)
    for par in range(2):
        for c in range(cfg.ncores):
            nc.sync.dma_start(a2a_in[par][c, :, NHALF:NHP], zcols[:])

    # All remaining SBUF/PSUM pools go on an inner stack that is closed
    # before build_body returns, so the persistent singles can then be
    # freed in proper LIFO order (their leaked-closure GC release would
    # otherwise interleave with the outer ctx pools).
    from contextlib import ExitStack
    inner = ExitStack()

    # weights
    wpool = inner.enter_context(tc.tile_pool(name="wts", bufs=1))
    wq_sb = wpool.tile([128, DCH, 128], bf16)
    wk_sb = wpool.tile([128, DCH, 128], bf16)
    wv_sb = wpool.tile([128, DCH, 128], bf16)
    nc.sync.dma_start(wq_sb[:], wq.rearrange("(c p) m -> p c m", p=128))
    nc.sync.dma_start(wk_sb[:], wk.rearrange("(c p) m -> p c m", p=128))
    nc.sync.dma_start(wv_sb[:], wv.rearrange("(c p) m -> p c m", p=128))
    wo_sb = wpool.tile([128, HCH, cfg.D], bf16)
    svec = wpool.tile([128, HCH], f32)
    svec_bf = wpool.tile([128, HCH], bf16)

    # pools
    xtp = inner.enter_context(tc.tile_pool(name="xt", bufs=2))
    vtp = inner.enter_context(tc.tile_pool(name="vt", bufs=2))
    epool = inner.enter_context(tc.tile_pool(name="e", bufs=4))
    stg = inner.enter_context(tc.tile_pool(name="stg", bufs=2))
    nrmp = inner.enter_context(tc.tile_pool(name="nrm", bufs=2))
    gp = inner.enter_context(tc.tile_pool(name="g", bufs=1))
    poutp = inner.enter_context(tc.tile_pool(name="pout", bufs=3))
    sps = inner.enter_context(tc.tile_pool(name="sps", bufs=2, space="PSUM"))
    ops = inner.enter_context(tc.tile_pool(name="ops", bufs=1, space="PSUM"))
    wps = inner.enter_context(tc.tile_pool(name="wps", bufs=2, space="PSUM"))

    def emit_proj_pieces(rc):
        """QKV projections + V transpose for one 512-row chunk, split into
        small closures that the attention k-loop interleaves so the PE keeps
        streaming while ACT works through the exps."""
        r0 = rc * TQ
        b_idx = r0 // cfg.T
        kt0 = (r0 % cfg.T) // 128
        njt = TQ // 128
        st = {}
        pieces = []

        def p_load():
            st["xt"] = xtp.tile([128, DCH, TQ], bf16, tag="xt", name="xt")
            for d in range(DCH):
                nc.sync.dma_start(st["xt"][:, d, :],
                                  xt_d[ts(d, 128), r0:r0 + TQ])
        pieces.append(p_load)

        def mk_mm(w_sb, d0):
            def p():
                if d0 == 0:
                    st["ps"] = wps.tile([128, TQ], f32, tag="w", name="prj_ps")
                for d in range(d0, d0 + 2):
                    nc.tensor.matmul(st["ps"][:], w_sb[:, d, :],
                                     st["xt"][:, d, :],
                                     start=(d == 0), stop=(d == DCH - 1))
            return p

        def mk_out(dest_fn):
            def p():
                dest_fn(st["ps"])
            return p

        def qk_out(dest, b_sb):
            def f(ps):
                nc.vector.tensor_scalar(out=dest[:, r0:r0 + TQ], in0=ps[:],
                                        scalar1=b_sb[:, 0:1], scalar2=None,
                                        op0=ALU.add)
            return f

        def v_out(ps):
            st["vt"] = vtp.tile([128, TQ], bf16, tag="vt", name="vt")
            nc.vector.tensor_scalar(out=st["vt"][:], in0=ps[:],
                                    scalar1=bv_sb[:, 0:1], scalar2=None,
                                    op0=ALU.add)

        for w_sb, out_fn in ((wq_sb, qk_out(qt_all, bq_sb)),
                             (wk_sb, qk_out(kt_all, bk_sb)),
                             (wv_sb, v_out)):
            for d0 in range(0, DCH, 2):
                pieces.append(mk_mm(w_sb, d0))
            pieces.append(mk_out(out_fn))

        def mk_tr(hl, j):
            def p():
                hs = slice(hl * 64, (hl + 1) * 64)
                vp = wps.tile([128, 64], bf16, tag="w", name="vp")
                nc.tensor.transpose(vp[:], st["vt"][hs, ts(j, 128)],
                                    ident_bf[hs, hs])
                nc.vector.tensor_copy(vaug[:, b_idx, kt0 + j, hl, 0:64],
                                      vp[:])
            return p
        for hl in range(2):
            for j in range(njt):
                pieces.append(mk_tr(hl, j))
        return pieces

    def emit_attn(cc, bg=None, fast_drain=False):
        b, qc = divmod(cc, NQC)
        c0 = b * cfg.T + qc * TQ
        par, ci = cc % 2, cc // 2
        o_ps = [ops.tile([65, TQ], f32, tag=f"o{hl}", name=f"o_ps{hl}")
                for hl in range(2)]

        def flush_o(te, e_tile):
            for hl in range(2):
                nc.tensor.matmul(o_ps[hl][:], vaug[:, b, te, hl, 0:65],
                                 e_tile[:, ts(hl, TQ)],
                                 start=(te == 0), stop=(te == NKT - 1))

        prev_e = None
        for t in range(NKT):
            k0 = b * cfg.T + t * 128
            s_pair = sps.tile([128, 2 * TQ], f32, tag="s", name="s_pair")
            for hl in range(2):
                hs = slice(hl * 64, (hl + 1) * 64)
                nc.tensor.matmul(s_pair[:, ts(hl, TQ)],
                                 kt_all[hs, k0:k0 + 128],
                                 qt_all[hs, c0:c0 + TQ],
                                 start=True, stop=True)
            if prev_e is not None:
                flush_o(t - 1, prev_e)
            e_pair = epool.tile([128, 2 * TQ], bf16, tag="e", name="e_pair")
            nc.scalar.activation(e_pair[:], s_pair[:], AF.Exp, scale=0.125)
            prev_e = e_pair
            if bg:
                bg.popleft()()
            if len(bg) > 12:
                bg.popleft()()
        flush_o(NKT - 1, prev_e)

        # drain: unnormalized O to SBUF, sums row + sumsq row to DRAM
        nc.vector.tensor_copy(o_all[0:64, c0:c0 + TQ], o_ps[0][0:64, :])
        nc.vector.tensor_copy(o_all[64:128, c0:c0 + TQ], o_ps[1][0:64, :])
        srow2 = None
        if fast_drain:
            srow2 = stg.tile([33, TQ], f32, tag="srow2", name="srow2")
            nc.vector.memset(srow2[:], 1.0)
            for hl in range(2):
                nc.vector.tensor_copy(srow2[32 * hl:32 * hl + 1, :],
                                      o_ps[hl][64:65, :])
        else:
            for hl in range(2):
                srow = stg.tile([1, TQ], f32, tag=f"srow{hl}", name="srow")
                nc.vector.tensor_copy(srow[:], o_ps[hl][64:65, :])
                nc.sync.dma_start(sum_dram[hl, par, ci, :][None, :],
                                  srow[0:1, :])
        sq = stg.tile([128, TQ], bf16, tag="sq", name="sq")
        nc.vector.tensor_tensor(out=sq[:], in0=o_all[:, c0:c0 + TQ],
                                in1=o_all[:, c0:c0 + TQ], op=ALU.mult)
        nqs33 = None
        if cc == cfg.NCH - 1:
            # final chunk: sumsq straight into rows 0/32 (zeros elsewhere)
            # and kept on-chip for the tail
            nq33 = wps.tile([33, TQ], f32, tag="w", name="nq33")
            nc.tensor.matmul(nq33[:], ones2_33[:], sq[:], start=True,
                             stop=True)
            nqs33 = stg.tile([33, TQ], f32, tag="nqs33", name="nqs33")
            nc.vector.tensor_copy(nqs33[:], nq33[:])
        else:
            nq = wps.tile([2, TQ], f32, tag="w", name="nq")
            nc.tensor.matmul(nq[:], ones2[:], sq[:], start=True, stop=True)
            nqs = stg.tile([2, TQ], f32, tag="nqs", name="nqs")
            nc.vector.tensor_copy(nqs[:], nq[:])
            for hl in range(2):
                nc.sync.dma_start(nrm_dram[hl, par, ci, :][None, :],
                                  nqs[hl:hl + 1, :])
        return srow2, nqs33

    def emit_normalize(par, cis):
        """Batched reciprocal + o normalization + A2A staging for the given
        chunks (ci list, contiguous) of parity par, via the DRAM bounce."""
        ci_lo, ci_hi = cis[0], cis[-1] + 1
        ncc = ci_hi - ci_lo
        pcol = ncc * TQ // 64
        ssb = nrmp.tile([128, pcol], f32, tag="ssb", name="ssb")
        for hl in range(2):
            nc.sync.dma_start(
                ssb[hl * 64:(hl + 1) * 64, :],
                sum_dram[hl, par, ci_lo:ci_hi, :]
                .rearrange("c q -> (c q)").rearrange("(p n) -> p n", p=64))
        rcp = nrmp.tile([128, pcol], f32, tag="rcp", name="rcp")
        nc.vector.reciprocal(rcp[:], ssb[:])
        for hl in range(2):
            nc.sync.dma_start(
                rec_dram[hl, par, ci_lo:ci_hi, :]
                .rearrange("c q -> (c q)").rearrange("(p n) -> p n", p=64),
                rcp[hl * 64:(hl + 1) * 64, :])
        for ci in range(ci_lo, ci_hi):
            cc2 = ci * 2 + par
            cb = cc2 * TQ
            rb = nrmp.tile([128, TQ], f32, tag="rb", name="rb")
            for hl in range(2):
                nc.sync.dma_start(
                    rb[hl * 64:(hl + 1) * 64, :],
                    rec_dram[hl, par, ci, :][None, :]
                    .to_broadcast((64, TQ)))
            nc.vector.tensor_tensor(out=o_all[:, cb:cb + TQ],
                                    in0=o_all[:, cb:cb + TQ], in1=rb[:],
                                    op=ALU.mult)
            nc.sync.dma_start(a2a_in[par][ci][:, 0:NHALF],
                              o_all[:, cb:cb + TQ])

    def emit_normalize_fast(par, ci, srow2):
        """Latency-lean normalize for the final batch: reciprocal on-chip
        from the drain's sums rows, partition-broadcast via a PE rank-1
        outer product (ones x recip row) into PSUM — no DRAM round trips
        on the critical path (rec_dram still written for the tail norms)."""
        cc2 = ci * 2 + par
        cb = cc2 * TQ
        rcp2 = nrmp.tile([33, TQ], f32, tag="rcp2", name="rcp2")
        nc.vector.reciprocal(rcp2[:], srow2[:])
        last = ci * 2 + par == cfg.NCH - 1
        for hl in range(2):
            # the final chunk's rec goes to a scratch row so the deferred
            # norms' rcb load never depends on it
            dst = (rec15_dram[hl, :][None, :] if last
                   else rec_dram[hl, par, ci, :][None, :])
            nc.sync.dma_start(dst, rcp2[32 * hl:32 * hl + 1, :])
        rb = nrmp.tile([128, TQ], f32, tag="rb", name="rb")
        for hl in range(2):
            src_row = (rec15_dram[hl, :][None, :] if last
                       else rec_dram[hl, par, ci, :][None, :])
            nc.sync.dma_start(rb[hl * 64:(hl + 1) * 64, :],
                              src_row.to_broadcast((64, TQ)))
        nc.vector.tensor_tensor(out=o_all[:, cb:cb + TQ],
                                in0=o_all[:, cb:cb + TQ], in1=rb[:],
                                op=ALU.mult)
        nc.sync.dma_start(a2a_in[par][ci][:, 0:NHALF],
                          o_all[:, cb:cb + TQ])
        return rcp2

    def emit_a2a(par):
        nc.gpsimd.collective_compute(
            "AllToAll", ALU.bypass,
            replica_groups=[list(range(cfg.ncores))],
            ins=[a2a_in[par][:].opt()], outs=[a2a_out[par][:].opt()])

    # ---------------- emission order ----------------
    from collections import deque
    for rc in range(NQC):           # prologue: project batch 0
        for p in emit_proj_pieces(rc):
            p()
    for b in range(cfg.B):
        bg = deque()
        if b < cfg.B - 1:
            for i in range(NQC):
                bg.extend(emit_proj_pieces((b + 1) * NQC + i))
        qcs = [0, 1, 2, 3] if b < cfg.B - 1 else [0, 2, 1, 3]
        for i, qc in enumerate(qcs):
            fast = b == cfg.B - 1
            srow2, nqs33 = emit_attn(b * NQC + qc, bg, fast_drain=fast)
            if fast:
                # per-chunk latency-lean normalize so each A2A piece fires
                # as soon as its last chunk drains
                par, ci = qc % 2, (b * NQC + qc) // 2
                rcp2_last = emit_normalize_fast(par, ci, srow2)
                if qc == 2:
                    emit_a2a(0)
                if nqs33 is not None:
                    nqs33_last = nqs33
        while bg:
            bg.popleft()()
        if b < cfg.B - 1:
            emit_normalize(0, [2 * b, 2 * b + 1])
            emit_normalize(1, [2 * b, 2 * b + 1])
        if b == cfg.B - 2:
            # 2 MB wo load during b3's attention window (Sync has slack)
            nc.sync.dma_start(wo_sb[:],
                              wo.rearrange("(c p) m -> p c m", p=128))
    # prefetch the sqrt ACT table right after the last exp; the junk DMA
    # keeps the dummy activation alive through dead-code elimination
    nc.scalar.activation(scr[:], ones_f32[:], AF.Sqrt)
    nc.sync.dma_start(ag_in[2:3][:, None], scr[0:1, 0:1])

    # ---------------- tail: norms -> s -> AllGather ---------------------
    g_sbs = [gp.tile([128, HCH, NHALF], bf16, tag="g", bufs=2,
                     name=f"g_sb{h}") for h in range(2)]

    pcol = 2 * NCP * TQ // 64       # all chunks, both parities, per head
    nsb = nrmp.tile([128, pcol], f32, tag="nsb", name="nsb")
    rcb = nrmp.tile([128, pcol], f32, tag="rcb", name="rcb")
    for hl in range(2):
        nc.sync.dma_start(
            nsb[hl * 64:(hl + 1) * 64, :],
            nrm_dram[hl].rearrange("a c q -> (a c q)")
            .rearrange("(p n) -> p n", p=64))
        nc.sync.dma_start(
            rcb[hl * 64:(hl + 1) * 64, :],
            rec_dram[hl].rearrange("a c q -> (a c q)")
            .rearrange("(p n) -> p n", p=64))
    nrt = nrmp.tile([128, pcol], f32, tag="nrt", name="nrt")
    nc.scalar.activation(nrt[:], nsb[:], AF.Sqrt)
    nc.vector.tensor_tensor(out=nrt[:], in0=nrt[:], in1=rcb[:], op=ALU.mult)
    rsum = nrmp.tile([128, 1], f32, tag="rsum", name="rsum")
    nc.vector.tensor_reduce(rsum[:], nrt[:], axis=mybir.AxisListType.X,
                            op=ALU.add)
    rsum_bf = nrmp.tile([128, 1], bf16, tag="rsumb", name="rsum_bf")
    nc.vector.tensor_copy(rsum_bf[:], rsum[:])
    # final chunk's contribution, entirely on-chip (rows 0/32)
    nrt33 = nrmp.tile([33, TQ], f32, tag="nrt33", name="nrt33")
    nc.scalar.activation(nrt33[:], nqs33_last[:], AF.Sqrt)
    nc.vector.tensor_tensor(out=nrt33[:], in0=nrt33[:], in1=rcp2_last[:],
                            op=ALU.mult)
    r33 = nrmp.tile([33, 1], f32, tag="r33", name="r33")
    nc.vector.tensor_reduce(r33[:], nrt33[:], axis=mybir.AxisListType.X,
                            op=ALU.add)
    r33_bf = nrmp.tile([33, 1], bf16, tag="r33b", name="r33_bf")
    nc.vector.tensor_copy(r33_bf[:], r33[:])
    ntot = wps.tile([2, 1], f32, tag="w", name="ntot")
    nc.tensor.matmul(ntot[:], ones2[:], rsum_bf[:], start=True, stop=False)
    nc.tensor.matmul(ntot[:], ones33[:], r33_bf[:], start=False, stop=True)
    s2 = nrmp.tile([2, 1], f32, tag="s2", name="s2")
    nc.vector.tensor_scalar(out=s2[:], in0=ntot[:], scalar1=1.0 / RT,
                            scalar2=1e-5, op0=ALU.mult, op1=ALU.max)
    nc.vector.reciprocal(s2[:], s2[:])
    s2b = nrmp.tile([2, 1], bf16, tag="s2b", name="s2b")
    nc.vector.tensor_copy(s2b[:], s2[:])
    for c in range(cfg.ncores):
        nc.sync.dma_start(a2a_in[1][c, 0:2, NHALF:NHALF + 1], s2b[0:2, 0:1])
    emit_a2a(1)
    for hl in range(2):
        nc.sync.dma_start(
            svec_bf[hl * 64:(hl + 1) * 64, :],
            a2a_out[1][:, hl, NHALF][None, :].to_broadcast((64, HCH)))
    nc.vector.tensor_copy(svec[:], svec_bf[:])
    # g piece 0 load last on the Sync queue (its a2a0 wait must not block
    # the norms chain; consumers need svec anyway)
    nc.sync.dma_start(g_sbs[0][:],
                      a2a_out[0][:, :, 0:NHALF].rearrange("c p q -> p c q"))
    # g piece 1 on the Scalar HWDGE queue, after the sqrts
    nc.scalar.dma_start(g_sbs[1][:],
                        a2a_out[1][:, :, 0:NHALF].rearrange("c p q -> p c q"))

    for par in range(2):
        g_sb = g_sbs[par]
        for j in range(HCH):
            nc.vector.tensor_scalar(out=g_sb[:, j, :], in0=g_sb[:, j, :],
                                    scalar1=svec[:, j:j + 1], scalar2=None,
                                    op0=ALU.mult)
        for dsub in range(DCH):
            ps = wps.tile([128, NHALF], f32, tag="w", name="p_ps")
            for j in range(HCH):
                nc.tensor.matmul(ps[:], wo_sb[:, j, ts(dsub, 128)],
                                 g_sb[:, j, :],
                                 start=(j == 0), stop=(j == HCH - 1))
            po = poutp.tile([128, NHALF], f32, tag="po", name="po")
            nc.vector.tensor_scalar(out=po[:], in0=ps[:],
                                    scalar1=bo_sb[:, dsub:dsub + 1],
                                    scalar2=None, op0=ALU.add)
            nc.sync.dma_start(
                out[ts(dsub, 128), par * NHALF:(par + 1) * NHALF], po[:])

    # close all inner pools, then release the persistent singles in LIFO
    # order so the pool stack stays consistent.
    inner.close()
    free_va()
    free_kt()
    free_qt()
    free_oa()


def build_nc(cfg, compile=True):
    nc = bacc.Bacc("TRN2", target_bir_lowering=False, debug=False,
                   enable_asserts=False, num_devices=cfg.ncores)
    xt_d = nc.dram_tensor("xt", [cfg.D, cfg.RT], bf16, kind="ExternalInput").ap()
    wq = nc.dram_tensor("wq", [cfg.D, 128], bf16, kind="ExternalInput").ap()
    wk = nc.dram_tensor("wk", [cfg.D, 128], bf16, kind="ExternalInput").ap()
    wv = nc.dram_tensor("wv", [cfg.D, 128], bf16, kind="ExternalInput").ap()
    bq = nc.dram_tensor("bq", [128], f32, kind="ExternalInput").ap()
    bk = nc.dram_tensor("bk", [128], f32, kind="ExternalInput").ap()
    bv = nc.dram_tensor("bv", [128], f32, kind="ExternalInput").ap()
    wo = nc.dram_tensor("wo", [128 * cfg.ncores, cfg.D], bf16,
                        kind="ExternalInput").ap()
    bo = nc.dram_tensor("bo", [cfg.D], f32, kind="ExternalInput").ap()
    out = nc.dram_tensor("out", [cfg.D, cfg.RSLC], f32, kind="ExternalOutput").ap()
    from contextlib import ExitStack
    with tile.TileContext(nc) as tc, ExitStack() as ctx:
        build_body(ctx, tc, cfg, xt_d, wq, wk, wv, bq, bk, bv, wo, bo, out)
    if compile:
        nc.compile()
    return nc


def make_in_maps(cfg, inputs, H_total=None):
    """Host-side sharding: per-core input dicts."""
    H_tot = H_total or (2 * cfg.ncores)
    XT = np.ascontiguousarray(
        np.asarray(inputs["hidden_states"], np.float32)
        .reshape(cfg.RT, cfg.D).T
    ).astype(BF16NP)
    gate_clip = np.clip(np.asarray(inputs["gate"], np.float32), 0.0, 1.0)
    Wo = np.asarray(inputs["Wo"], np.float32)
    bo = np.asarray(inputs["bo"], np.float32)
    wo_all = np.ascontiguousarray(np.concatenate(
        [Wo[h] * (gate_clip[h] / H_tot) for h in range(H_tot)],
        axis=0)).astype(BF16NP)
    bo_sum = (bo * (gate_clip[:, None] / H_tot)).sum(axis=0).astype(np.float32)
    in_maps = []
    for c in range(cfg.ncores):
        h0, h1 = 2 * c, 2 * c + 1
        m = {
            "xt": XT,
            "wq": np.concatenate([inputs["Wq"][h0], inputs["Wq"][h1]], axis=1,
                                 dtype=np.float32).astype(BF16NP),
            "wk": np.concatenate([inputs["Wk"][h0], inputs["Wk"][h1]], axis=1,
                                 dtype=np.float32).astype(BF16NP),
            "wv": np.concatenate([inputs["Wv"][h0], inputs["Wv"][h1]], axis=1,
                                 dtype=np.float32).astype(BF16NP),
            "bq": np.concatenate([inputs["bq"][h0], inputs["bq"][h1]],
                                 dtype=np.float32),
            "bk": np.concatenate([inputs["bk"][h0], inputs["bk"][h1]],
                                 dtype=np.float32),
            "bv": np.concatenate([inputs["bv"][h0], inputs["bv"][h1]],
                                 dtype=np.float32),
            "wo": wo_all,
            "bo": bo_sum,
        }
        in_maps.append(m)
    return in_maps


def gather_out(cfg, results):
    """results: list of per-core out_maps -> full [B, T, D]."""
    parts = [np.asarray(r["out"]).T for r in results]  # each [RSLC, D]
    return np.concatenate(parts, axis=0).reshape(cfg.B, cfg.T, cfg.D)


_COMPILED = {}


def kernel(**inputs) -> np.ndarray:
    cfg = Cfg()
    key = "full"
    if key not in _COMPILED:
        _COMPILED[key] = build_nc(cfg)
    nc = _COMPILED[key]
    in_maps = make_in_maps(cfg, inputs)
    last_exc = None
    for _attempt in range(3):
        try:
            res = bass_utils.run_bass_kernel_spmd(
                nc, in_maps, core_ids=list(range(cfg.ncores)))
            return gather_out(cfg, res.results)
        except Exception as e:  # transient NRT_EXEC_UNIT_UNRECOVERABLE faults
            last_exc = e
    raise last_exc


if __name__ == "__main__":
    import reference
    inputs = {k: np.asarray(v) for k, v in reference.setup_inputs().items()}
    out = kernel(**inputs)
    exp = np.asarray(reference.reference(**inputs))
    rel = np.linalg.norm(out - exp) / np.linalg.norm(exp)
    print("Relative error:", rel)
